# revision 6
# baseline (speedup 1.0000x reference)
"""Multi-head attention (B=8, S=1024, D=1024, H=16) on 8 Trainium2 NeuronCores.

Sharding: data-parallel over batch - core b computes batch element b end to
end (no collectives). Weights are replicated to every core.

Design (HW-measured ~2-4x faster than the v1 baseline; the dominant HW
bottleneck turned out to be DMA-queue serialization, not PE/ACT):
  - weight DMAs double-buffered and spread over the three DGE queues
    (WQ/WK on ACT hwdge, x/WV/WO + output stores on SP hwdge, normalize
    spill traffic on the gpsimd SWDGE queue)
  - Q^T/K^T and Y^T stored bf16 (same PE rate at N=512, half SBUF); the
    output GEMM runs pure-bf16 (WO converted once, FWL-eligible weight
    loads, bf16 bias/fixup rows keep the PSUM group dtype-pure)
  - attention query-chunk-outer: score tile [128,1024] holds both heads of
    a pair, ONE exp activation per (pair, kk); AV accumulates per head with
    the V-hat ones column producing the softmax row-sums at partition 64
  - softmax normalization fully decoupled: AV staged to SBUF (PSUM freed),
    reciprocal and (1-mask) multiply run at partition 64, the broadcast is
    a K=1 matmul whose contraction row IS partition 64 (no partition-shift
    DMA), both heads' reciprocal and mask-multiply run as single batched
    [1,2,512] DVE ops, and the chain is deferred into the NEXT pair's kk
    loop so the in-order PE queue never head-of-line blocks; per-partition
    Q/K biases are applied by DVE tensor_scalar, keeping ACT exp-only
  - output GEMM split in [128,512] units, interleaved into the second query
    chunk's attention stream to fill ACT-paced PE gaps
  - masked queries: reference softmax is uniform there, so those Y^T columns
    are zeroed via (1-mask) and the output GEMM re-adds
    mask_q x (mean_k V @ WO) + bO as K=1 matmuls

`reps` repeats the whole body inside one NEFF for marginal-time HW
measurement (no NTFF profiling exists through the axon tunnel).
"""

import numpy as np

import concourse.bass as bass
import concourse.mybir as mybir
from concourse.tile import TileContext
from concourse.bass_utils import run_bass_kernel_spmd

AF = mybir.ActivationFunctionType
F32 = mybir.dt.float32
F32R = mybir.dt.float32r
BF16 = mybir.dt.bfloat16

B, S, D, H = 8, 1024, 1024, 16
DH = D // H          # 64
P = 128
ST = S // P          # 8 s-tiles
DT = D // P          # 8 d-tiles
NEG = -30.0
N_CORES = 8

_nop_counter = [0]


def _split_multi_waits(nc):
    for bb in nc.main_func.blocks:
        raw = bb.bb if hasattr(bb, "bb") else bb
        changed = False
        new_list = []
        for ins in raw.instructions:
            si = ins.sync_info
            waits = list(si.on_wait) if si is not None else []
            if len(waits) > 1:
                changed = True
                for w in waits[:-1]:
                    _nop_counter[0] += 1
                    nop = mybir.InstNoOp(
                        name=f"legal_wait_nop_{_nop_counter[0]}", engine=ins.engine
                    )
                    nop.sync_info = mybir.SyncInfo(on_wait=[w], on_update=[])
                    new_list.append(nop)
                ins.sync_info = mybir.SyncInfo(
                    on_wait=[waits[-1]], on_update=list(si.on_update)
                )
            new_list.append(ins)
        if changed:
            raw.instructions = new_list


def _make_identity(nc, ident_f32):
    nc.gpsimd.memset(ident_f32[:], 0.0)
    nc.gpsimd.affine_select(
        out=ident_f32[:],
        in_=ident_f32[:],
        compare_op=mybir.AluOpType.not_equal,
        fill=1.0,
        base=0,
        pattern=[[-1, P]],
        channel_multiplier=1,
    )


def _build(reps=1):
    from contextlib import ExitStack

    nc = bass.Bass("TRN2", num_devices=N_CORES)

    x_d = nc.dram_tensor("x", [S, D], F32, kind="ExternalInput")
    wq_d = nc.dram_tensor("WQ", [D, D], F32R, kind="ExternalInput")
    wk_d = nc.dram_tensor("WK", [D, D], F32R, kind="ExternalInput")
    wv_d = nc.dram_tensor("WV", [D, D], F32R, kind="ExternalInput")
    wo_d = nc.dram_tensor("WO", [D, D], F32R, kind="ExternalInput")
    bq_d = nc.dram_tensor("bQ", [D], F32, kind="ExternalInput")
    bk_d = nc.dram_tensor("bK", [D], F32, kind="ExternalInput")
    bv_d = nc.dram_tensor("bV", [D], F32R, kind="ExternalInput")
    bo_d = nc.dram_tensor("bO", [D], F32R, kind="ExternalInput")
    mv_d = nc.dram_tensor("mvec", [S], F32, kind="ExternalInput")     # -30*mask
    om_d = nc.dram_tensor("onem", [S], F32, kind="ExternalInput")     # 1-mask
    mf_d = nc.dram_tensor("maskf", [S], F32R, kind="ExternalInput")   # mask
    out_d = nc.dram_tensor("out", [S, D], F32, kind="ExternalOutput")

    with TileContext(nc) as tc:
        with tc.tile_pool(name="misc", bufs=1) as misc:
            ident = misc.tile([P, P], F32, tag="ident")
            _make_identity(nc, ident)
            onesf = misc.tile([1, P], F32, tag="onesf")
            nc.vector.memset(onesf[:], 1.0)
            onesr = misc.tile([1, P], F32R, tag="onesr")
            nc.vector.tensor_copy(onesr[:], onesf[:])
            onek = misc.tile([P, 1], BF16, tag="onek")
            nc.vector.memset(onek[:], 1.0 / 1024.0)

            bq_sb = misc.tile([P, DT], F32, tag="bq")
            nc.gpsimd.dma_start(bq_sb[:], bq_d.rearrange("(c p) -> p c", p=P))
            bk_sb = misc.tile([P, DT], F32, tag="bk")
            nc.gpsimd.dma_start(bk_sb[:], bk_d.rearrange("(c p) -> p c", p=P))
            bv_row = misc.tile([1, D], F32R, tag="bv")
            nc.gpsimd.dma_start(bv_row[:], bv_d[None, :])
            bo_row = misc.tile([1, D], F32R, tag="bo")
            nc.gpsimd.dma_start(bo_row[:], bo_d[None, :])
            mvec = misc.tile([P, ST], F32, tag="mvec")
            nc.gpsimd.dma_start(mvec[:], mv_d.rearrange("(t p) -> p t", p=P))
            onem_row = misc.tile([1, S], F32, tag="onem")
            nc.gpsimd.dma_start(onem_row[:], om_d[None, :])
            mf_row = misc.tile([1, S], F32R, tag="maskf")
            nc.gpsimd.dma_start(mf_row[:], mf_d[None, :])

            ones64 = misc.tile([DH + 1, P], F32R, tag="ones64")
            nc.vector.tensor_copy(ones64[DH:DH + 1, :], onesf[:])
            omq64 = misc.tile([DH + 1, 2, S], F32, tag="omq64")
            nc.gpsimd.dma_start(omq64[DH:DH + 1, 0, :], onem_row[0:1, :])
            nc.gpsimd.dma_start(omq64[DH:DH + 1, 1, :], onem_row[0:1, :])

            onesb = misc.tile([1, P], BF16, tag="onesb")
            nc.vector.memset(onesb[:], 1.0)
            bo_b = misc.tile([1, D], BF16, tag="bo_b")
            nc.vector.tensor_copy(bo_b[:], bo_row[:])
            mf_b = misc.tile([1, S], BF16, tag="mf_b")
            nc.vector.tensor_copy(mf_b[:], mf_row[:])

            ucol = misc.tile([P, DT], F32R, tag="ucol")
            w0_row = misc.tile([1, D], F32R, tag="w0")
            w0_b = misc.tile([1, D], BF16, tag="w0_b")

            for rep in range(reps):
                _emit_rep(nc, tc, ExitStack, locals())

    _split_multi_waits(nc)
    return nc


def _emit_rep(nc, tc, ExitStack, env):
    ident = env["ident"]; onesr = env["onesr"]; onek = env["onek"]
    bq_sb = env["bq_sb"]; bk_sb = env["bk_sb"]
    bv_row = env["bv_row"]; bo_row = env["bo_row"]; mvec = env["mvec"]
    onem_row = env["onem_row"]; mf_row = env["mf_row"]
    ones64 = env["ones64"]; omq64 = env["omq64"]
    onesb = env["onesb"]; bo_b = env["bo_b"]; mf_b = env["mf_b"]
    w0_b = env["w0_b"]
    ucol = env["ucol"]; w0_row = env["w0_row"]
    x_d = env["x_d"]; wq_d = env["wq_d"]; wk_d = env["wk_d"]
    wv_d = env["wv_d"]; wo_d = env["wo_d"]; out_d = env["out_d"]

    with ExitStack() as es0:
        qkp = es0.enter_context(tc.tile_pool(name="qk", bufs=1))
        vhp = es0.enter_context(tc.tile_pool(name="vh", bufs=1))
        ytp = es0.enter_context(tc.tile_pool(name="yt", bufs=1))
        wp = es0.enter_context(tc.tile_pool(name="w", bufs=2))

        qt = qkp.tile([P, DT, S], BF16, tag="qt")
        kt = qkp.tile([P, DT, S], BF16, tag="kt")
        vhat = vhp.tile([P, ST, H, DH + 1], BF16, tag="vhat")
        nc.vector.memset(vhat[:, :, :, DH], 1.0)
        yt = ytp.tile([P, DT, S], BF16, tag="yt")

        # weight loads: WQ/WK on the ACT hwdge queue; x/WV/WO on SP
        wq_sb = wp.tile([P, DT, D], F32R, tag="w", name="wq_sb")
        nc.scalar.dma_start(wq_sb[:], wq_d.rearrange("(t p) n -> p t n", p=P))
        wk_sb = wp.tile([P, DT, D], F32R, tag="w", name="wk_sb")
        nc.scalar.dma_start(wk_sb[:], wk_d.rearrange("(t p) n -> p t n", p=P))

        # ---- phases A-D: x^T, Q^T, K^T, V ----
        with ExitStack() as es1:
            xtp = es1.enter_context(tc.tile_pool(name="xt", bufs=1))
            xin = es1.enter_context(tc.tile_pool(name="xin", bufs=1))
            pst = es1.enter_context(
                tc.tile_pool(name="pst", bufs=3, space="PSUM"))
            trp = es1.enter_context(
                tc.tile_pool(name="trp", bufs=2, space="PSUM"))

            xT = xtp.tile([P, DT, S], F32R, tag="xT")
            for i in range(ST):
                x_t = xin.tile([P, D], F32, tag="x")
                nc.sync.dma_start(x_t[:], x_d[i * P:(i + 1) * P, :])
                for j in range(DT):
                    tp = trp.tile([P, P], F32, tag="tr")
                    nc.tensor.transpose(
                        tp[:], x_t[:, j * P:(j + 1) * P], ident[:])
                    nc.vector.tensor_copy(
                        xT[:, j, i * P:(i + 1) * P], tp[:])

            wv_sb = wp.tile([P, DT, D], F32R, tag="w", name="wv_sb")
            nc.sync.dma_start(
                wv_sb[:], wv_d.rearrange("(t p) n -> p t n", p=P))
            wo_sb = wp.tile([P, DT, D], F32R, tag="w", name="wo_sb")
            nc.sync.dma_start(
                wo_sb[:], wo_d.rearrange("(t p) n -> p t n", p=P))

            # B/C: Q^T and K^T (ACT applies the per-partition bias, bf16 out)
            for (w_sb, b_sb, dst) in ((wq_sb, bq_sb, qt), (wk_sb, bk_sb, kt)):
                for c in range(DT):
                    ps = pst.tile([P, S], F32, tag="proj")
                    for t in range(DT):
                        for h2 in range(2):
                            nc.tensor.matmul(
                                ps[:, h2 * 512:(h2 + 1) * 512],
                                w_sb[:, t, c * P:(c + 1) * P],
                                xT[:, t, h2 * 512:(h2 + 1) * 512],
                                start=(t == 0), stop=(t == DT - 1))
                    nc.vector.tensor_scalar_add(
                        dst[:, c, :], ps[:], b_sb[:, c:c + 1])

            # D: V with bias folded in as K=1 matmul; bf16 head-strided store
            for c in range(ST):
                ps = pst.tile([P, S], F32, tag="proj")
                for t in range(DT):
                    for h2 in range(2):
                        nc.tensor.matmul(
                            ps[:, h2 * 512:(h2 + 1) * 512],
                            xT[:, t, c * P:(c + 1) * P],
                            wv_sb[:, t, h2 * 512:(h2 + 1) * 512],
                            start=(t == 0), stop=False)
                for h2 in range(2):
                    nc.tensor.matmul(
                        ps[:, h2 * 512:(h2 + 1) * 512],
                        onesr[0:1, 0:P],
                        bv_row[0:1, h2 * 512:(h2 + 1) * 512],
                        start=False, stop=True)
                nc.vector.tensor_copy(
                    vhat[:, c, :, 0:DH],
                    ps[:].rearrange("p (h e) -> p h e", h=H))

        # u = mean_k V, w0 = u @ WO (for the masked-query fixup)
        with tc.tile_pool(name="psu", bufs=2, space="PSUM") as psu, \
             tc.tile_pool(name="psw", bufs=1, space="PSUM") as psw:
            for t in range(DT):
                up = psu.tile([P, 1], F32, tag="u")
                for kk in range(ST):
                    nc.tensor.matmul(
                        up[0:DH, :], vhat[:, kk, 2 * t, 0:DH],
                        onek[:], start=(kk == 0), stop=(kk == ST - 1))
                for kk in range(ST):
                    nc.tensor.matmul(
                        up[DH:P, :], vhat[:, kk, 2 * t + 1, 0:DH],
                        onek[:], start=(kk == 0), stop=(kk == ST - 1),
                        tile_position=(0, DH))
                nc.vector.tensor_copy(ucol[:, t:t + 1], up[:])
            wps = psw.tile([1, D], F32, tag="w0ps")
            for t in range(DT):
                for h2 in range(2):
                    nc.tensor.matmul(
                        wps[0:1, h2 * 512:(h2 + 1) * 512],
                        ucol[:, t:t + 1],
                        wo_sb[:, t, h2 * 512:(h2 + 1) * 512],
                        start=(t == 0), stop=(t == DT - 1))
            nc.vector.tensor_copy(w0_row[:], wps[:])
            nc.vector.tensor_copy(w0_b[:], wps[:])

        # WO -> bf16 for the output GEMM (FWL-eligible weight loads)
        wobp = tc.tile_pool(name="wob", bufs=1)
        es0.callback(lambda: wobp.__exit__(None, None, None))
        wob_pool = wobp.__enter__()
        wob = wob_pool.tile([P, DT, D], BF16, tag="wob")
        for t in range(DT):
            nc.vector.tensor_copy(wob[:, t, :], wo_sb[:, t, :])

        # ---- phase E: attention + phase F: output GEMM, interleaved ----
        with ExitStack() as es2:
            stp = es2.enter_context(
                tc.tile_pool(name="stp", bufs=2, space="PSUM"))
            avp = es2.enter_context(
                tc.tile_pool(name="avp", bufs=2, space="PSUM"))
            fpo = es2.enter_context(
                tc.tile_pool(name="fpo", bufs=2, space="PSUM"))
            epool = es2.enter_context(tc.tile_pool(name="ep", bufs=3))
            avsb = es2.enter_context(tc.tile_pool(name="avsb", bufs=2))
            rcp = es2.enter_context(tc.tile_pool(name="rcp", bufs=1))
            scrp = es2.enter_context(tc.tile_pool(name="scr", bufs=1))
            outp = es2.enter_context(tc.tile_pool(name="outp", bufs=2))

            ob_cur = {}

            def emit_f_unit(c, h2):
                po = fpo.tile([P, 512], F32, tag="o")
                for t in range(DT):
                    nc.tensor.matmul(
                        po[:], yt[:, t, c * P:(c + 1) * P],
                        wob[:, t, h2 * 512:(h2 + 1) * 512],
                        start=(t == 0), stop=False)
                nc.tensor.matmul(
                    po[:], onesb[0:1, 0:P],
                    bo_b[0:1, h2 * 512:(h2 + 1) * 512],
                    start=False, stop=False)
                nc.tensor.matmul(
                    po[:], mf_b[0:1, c * P:(c + 1) * P],
                    w0_b[0:1, h2 * 512:(h2 + 1) * 512],
                    start=False, stop=True)
                if h2 == 0:
                    ob_cur[c] = outp.tile([P, S], F32, tag="osb",
                                          name=f"ob_{c}")
                ob = ob_cur[c]
                nc.vector.tensor_copy(
                    ob[:, h2 * 512:(h2 + 1) * 512], po[:])
                if h2 == 1:
                    nc.sync.dma_start(
                        out_d[c * P:(c + 1) * P, :], ob[:])

            # deferred normalize tail (v1-style ops): emitted mid-way through
            # the NEXT pair's kk loop so PE never head-of-line-blocks
            pending = [None]

            def emit_norm_tail(avs, pr, qs, qc):
                rc64 = rcp.tile([DH + 1, 2, 512], F32R, tag="rc64",
                                name=f"rc64_{qc}_{pr}")
                with nc.allow_low_precision(reason="softmax recip"):
                    nc.vector.reciprocal(
                        rc64[DH:DH + 1, :, :], avs[DH:DH + 1, :, :])
                nc.vector.tensor_tensor(
                    rc64[DH:DH + 1, :, :], rc64[DH:DH + 1, :, :],
                    omq64[DH:DH + 1, :, qs],
                    mybir.AluOpType.mult)
                for sub in range(2):
                    bc = fpo.tile([DH, 512], F32, tag="o",
                                  name=f"bc_{qc}_{pr}_{sub}")
                    nc.tensor.matmul(
                        bc[:], ones64[DH:DH + 1, 0:DH],
                        rc64[DH:DH + 1, sub, :],
                        start=True, stop=True)
                    if sub == 0:
                        nc.vector.tensor_tensor(
                            yt[0:DH, pr, qs], avs[0:DH, 0, :], bc[:],
                            mybir.AluOpType.mult)
                    else:
                        scr = scrp.tile([DH, 512], BF16, tag="scr",
                                        name=f"scr_{qc}_{pr}")
                        nc.vector.tensor_tensor(
                            scr[:], avs[0:DH, 1, :], bc[:],
                            mybir.AluOpType.mult)
                        nc.gpsimd.dma_start(yt[DH:P, pr, qs], scr[:])

            f_queue = []
            for qc in range(2):
                qs = slice(qc * 512, (qc + 1) * 512)
                for pr in range(H // 2):
                    av0 = avp.tile([DH + 1, 512], F32, tag="av",
                                   name=f"av0_{qc}_{pr}")
                    av1 = avp.tile([DH + 1, 512], F32, tag="av",
                                   name=f"av1_{qc}_{pr}")
                    for kk in range(ST):
                        st = stp.tile([P, S], F32, tag="st")
                        nc.tensor.matmul(
                            st[:, 0:512],
                            kt[0:DH, pr, kk * P:(kk + 1) * P],
                            qt[0:DH, pr, qs],
                            start=True, stop=True)
                        nc.tensor.matmul(
                            st[:, 512:1024],
                            kt[DH:P, pr, kk * P:(kk + 1) * P],
                            qt[DH:P, pr, qs],
                            start=True, stop=True)
                        e_t = epool.tile([P, S], BF16, tag="E",
                                         name=f"e_{qc}_{pr}_{kk}")
                        nc.scalar.activation(
                            e_t[:], st[:], AF.Exp,
                            bias=mvec[:, kk:kk + 1], scale=0.125)
                        nc.tensor.matmul(
                            av0[:], vhat[:, kk, 2 * pr, :],
                            e_t[:, 0:512],
                            start=(kk == 0), stop=(kk == ST - 1))
                        nc.tensor.matmul(
                            av1[:], vhat[:, kk, 2 * pr + 1, :],
                            e_t[:, 512:1024],
                            start=(kk == 0), stop=(kk == ST - 1))
                        if kk == 2:
                            if pending[0] is not None:
                                emit_norm_tail(*pending[0])
                                pending[0] = None
                            if f_queue:
                                emit_f_unit(*f_queue.pop(0))
                    # stage AV to SBUF, free PSUM
                    avs = avsb.tile([DH + 1, 2, 512], F32, tag="avs",
                                    name=f"avs_{qc}_{pr}")
                    nc.vector.tensor_copy(avs[:, 0, :], av0[:])
                    nc.vector.tensor_copy(avs[:, 1, :], av1[:])
                    pending[0] = (avs, pr, qs, qc)
                if qc == 0:
                    f_queue = [(c, h2) for c in range(4) for h2 in range(2)]
            if pending[0] is not None:
                emit_norm_tail(*pending[0])
                pending[0] = None
            for cu in f_queue:
                emit_f_unit(*cu)
            for c in range(4, 8):
                for h2 in range(2):
                    emit_f_unit(c, h2)


_cached = {}


def kernel(**inputs):
    ins = {k: np.asarray(v) for k, v in inputs.items()}
    x = ins["x"].astype(np.float32)            # [B, S, D]
    mask = ins["mask"].astype(bool)            # [B, S]
    if "nc" not in _cached:
        _cached["nc"] = _build()
    nc = _cached["nc"]

    mask_f = mask.astype(np.float32)
    weights = {k: np.ascontiguousarray(ins[k].astype(np.float32))
               for k in ("WQ", "WK", "WV", "WO", "bQ", "bK", "bV", "bO")}
    in_maps = []
    for b in range(N_CORES):
        m = dict(weights)
        m["x"] = np.ascontiguousarray(x[b])
        m["mvec"] = np.ascontiguousarray(NEG * mask_f[b])
        m["onem"] = np.ascontiguousarray(1.0 - mask_f[b])
        m["maskf"] = np.ascontiguousarray(mask_f[b])
        in_maps.append(m)

    res = run_bass_kernel_spmd(nc, in_maps, core_ids=list(range(N_CORES)))
    return np.stack([r["out"] for r in res.results], axis=0)


# revision 7
# speedup vs baseline: 5.2867x; 5.2867x over previous
"""Multi-head attention (B=8, S=1024, D=1024, H=16) on 8 Trainium2 NeuronCores.

Sharding: data-parallel over batch - core b computes batch element b end to
end (no collectives). Weights are replicated to every core.

Design (HW-measured ~2-4x faster than the v1 baseline; the dominant HW
bottleneck turned out to be DMA-queue serialization, not PE/ACT):
  - weight DMAs double-buffered and spread over the three DGE queues
    (WQ/WK on ACT hwdge, x/WV/WO + output stores on SP hwdge, normalize
    spill traffic on the gpsimd SWDGE queue)
  - Q^T/K^T and Y^T stored bf16 (same PE rate at N=512, half SBUF); the
    output GEMM runs pure-bf16 (WO converted once, FWL-eligible weight
    loads, bf16 bias/fixup rows keep the PSUM group dtype-pure)
  - attention query-chunk-outer: score tile [128,1024] holds both heads of
    a pair, ONE exp activation per (pair, kk); AV accumulates per head with
    the V-hat ones column producing the softmax row-sums at partition 64
  - softmax normalization fully decoupled: AV staged to SBUF (PSUM freed),
    reciprocal and (1-mask) multiply run at partition 64, the broadcast is
    a K=1 matmul whose contraction row IS partition 64 (no partition-shift
    DMA), both heads' reciprocal and mask-multiply run as single batched
    [1,2,512] DVE ops, and the chain is deferred into the NEXT pair's kk
    loop so the in-order PE queue never head-of-line blocks; per-partition
    Q/K biases are applied by DVE tensor_scalar, keeping ACT exp-only
  - output GEMM split in [128,512] units, interleaved into the second query
    chunk's attention stream to fill ACT-paced PE gaps
  - masked queries: reference softmax is uniform there, so those Y^T columns
    are zeroed via (1-mask) and the output GEMM re-adds
    mask_q x (mean_k V @ WO) + bO as K=1 matmuls

`reps` repeats the whole body inside one NEFF for marginal-time HW
measurement (no NTFF profiling exists through the axon tunnel).
"""

import numpy as np

import concourse.bass as bass
import concourse.mybir as mybir
from concourse.tile import TileContext
from concourse.bass_utils import run_bass_kernel_spmd

AF = mybir.ActivationFunctionType
F32 = mybir.dt.float32
F32R = mybir.dt.float32r
BF16 = mybir.dt.bfloat16

B, S, D, H = 8, 1024, 1024, 16
DH = D // H          # 64
P = 128
ST = S // P          # 8 s-tiles
DT = D // P          # 8 d-tiles
NEG = -30.0
N_CORES = 8

_nop_counter = [0]


def _split_multi_waits(nc):
    for bb in nc.main_func.blocks:
        raw = bb.bb if hasattr(bb, "bb") else bb
        changed = False
        new_list = []
        for ins in raw.instructions:
            si = ins.sync_info
            waits = list(si.on_wait) if si is not None else []
            if len(waits) > 1:
                changed = True
                for w in waits[:-1]:
                    _nop_counter[0] += 1
                    nop = mybir.InstNoOp(
                        name=f"legal_wait_nop_{_nop_counter[0]}", engine=ins.engine
                    )
                    nop.sync_info = mybir.SyncInfo(on_wait=[w], on_update=[])
                    new_list.append(nop)
                ins.sync_info = mybir.SyncInfo(
                    on_wait=[waits[-1]], on_update=list(si.on_update)
                )
            new_list.append(ins)
        if changed:
            raw.instructions = new_list


def _make_identity(nc, ident_f32):
    nc.gpsimd.memset(ident_f32[:], 0.0)
    nc.gpsimd.affine_select(
        out=ident_f32[:],
        in_=ident_f32[:],
        compare_op=mybir.AluOpType.not_equal,
        fill=1.0,
        base=0,
        pattern=[[-1, P]],
        channel_multiplier=1,
    )


def _build(reps=1):
    from contextlib import ExitStack

    nc = bass.Bass("TRN2", num_devices=N_CORES)

    x_d = nc.dram_tensor("x", [S, D], F32, kind="ExternalInput")
    wq_d = nc.dram_tensor("WQ", [D, D], F32R, kind="ExternalInput")
    wk_d = nc.dram_tensor("WK", [D, D], F32R, kind="ExternalInput")
    wv_d = nc.dram_tensor("WV", [D, D], F32R, kind="ExternalInput")
    wo_d = nc.dram_tensor("WO", [D, D], F32R, kind="ExternalInput")
    bq_d = nc.dram_tensor("bQ", [D], F32, kind="ExternalInput")
    bk_d = nc.dram_tensor("bK", [D], F32, kind="ExternalInput")
    bv_d = nc.dram_tensor("bV", [D], F32R, kind="ExternalInput")
    bo_d = nc.dram_tensor("bO", [D], F32R, kind="ExternalInput")
    mv_d = nc.dram_tensor("mvec", [S], F32, kind="ExternalInput")     # -30*mask
    om_d = nc.dram_tensor("onem", [S], F32, kind="ExternalInput")     # 1-mask
    mf_d = nc.dram_tensor("maskf", [S], F32R, kind="ExternalInput")   # mask
    out_d = nc.dram_tensor("out", [S, D], F32, kind="ExternalOutput")

    with TileContext(nc) as tc:
        with tc.tile_pool(name="misc", bufs=1) as misc:
            ident = misc.tile([P, P], F32, tag="ident")
            _make_identity(nc, ident)
            onesf = misc.tile([1, P], F32, tag="onesf")
            nc.vector.memset(onesf[:], 1.0)
            onesr = misc.tile([1, P], F32R, tag="onesr")
            nc.vector.tensor_copy(onesr[:], onesf[:])
            onek = misc.tile([P, 1], BF16, tag="onek")
            nc.vector.memset(onek[:], 1.0 / 1024.0)

            bq_sb = misc.tile([P, DT], F32, tag="bq")
            nc.gpsimd.dma_start(bq_sb[:], bq_d.rearrange("(c p) -> p c", p=P))
            bk_sb = misc.tile([P, DT], F32, tag="bk")
            nc.gpsimd.dma_start(bk_sb[:], bk_d.rearrange("(c p) -> p c", p=P))
            bv_row = misc.tile([1, D], F32R, tag="bv")
            nc.gpsimd.dma_start(bv_row[:], bv_d[None, :])
            bo_row = misc.tile([1, D], F32R, tag="bo")
            nc.gpsimd.dma_start(bo_row[:], bo_d[None, :])
            mvec = misc.tile([P, ST], F32, tag="mvec")
            nc.gpsimd.dma_start(mvec[:], mv_d.rearrange("(t p) -> p t", p=P))
            onem_row = misc.tile([1, S], F32, tag="onem")
            nc.gpsimd.dma_start(onem_row[:], om_d[None, :])
            mf_row = misc.tile([1, S], F32R, tag="maskf")
            nc.gpsimd.dma_start(mf_row[:], mf_d[None, :])

            ones64 = misc.tile([DH + 1, P], F32R, tag="ones64")
            nc.vector.tensor_copy(ones64[DH:DH + 1, :], onesf[:])
            omq64 = misc.tile([DH + 1, 2, S], F32, tag="omq64")
            nc.gpsimd.dma_start(omq64[DH:DH + 1, 0, :], onem_row[0:1, :])
            nc.gpsimd.dma_start(omq64[DH:DH + 1, 1, :], onem_row[0:1, :])

            onesb = misc.tile([1, P], BF16, tag="onesb")
            nc.vector.memset(onesb[:], 1.0)
            bo_b = misc.tile([1, D], BF16, tag="bo_b")
            nc.vector.tensor_copy(bo_b[:], bo_row[:])
            mf_b = misc.tile([1, S], BF16, tag="mf_b")
            nc.vector.tensor_copy(mf_b[:], mf_row[:])

            ucol = misc.tile([P, DT], F32R, tag="ucol")
            w0_row = misc.tile([1, D], F32R, tag="w0")
            w0_b = misc.tile([1, D], BF16, tag="w0_b")

            for rep in range(reps):
                _emit_rep(nc, tc, ExitStack, locals())

    _split_multi_waits(nc)
    return nc


def _emit_rep(nc, tc, ExitStack, env):
    ident = env["ident"]; onesr = env["onesr"]; onek = env["onek"]
    bq_sb = env["bq_sb"]; bk_sb = env["bk_sb"]
    bv_row = env["bv_row"]; bo_row = env["bo_row"]; mvec = env["mvec"]
    onem_row = env["onem_row"]; mf_row = env["mf_row"]
    ones64 = env["ones64"]; omq64 = env["omq64"]
    onesb = env["onesb"]; bo_b = env["bo_b"]; mf_b = env["mf_b"]
    w0_b = env["w0_b"]
    ucol = env["ucol"]; w0_row = env["w0_row"]
    x_d = env["x_d"]; wq_d = env["wq_d"]; wk_d = env["wk_d"]
    wv_d = env["wv_d"]; wo_d = env["wo_d"]; out_d = env["out_d"]

    with ExitStack() as es0:
        qkp = es0.enter_context(tc.tile_pool(name="qk", bufs=1))
        vhp = es0.enter_context(tc.tile_pool(name="vh", bufs=1))
        ytp = es0.enter_context(tc.tile_pool(name="yt", bufs=1))
        wp = es0.enter_context(tc.tile_pool(name="w", bufs=2))

        qt = qkp.tile([P, DT, S], BF16, tag="qt")
        kt = qkp.tile([P, DT, S], BF16, tag="kt")
        vhat = vhp.tile([P, ST, H, DH + 1], BF16, tag="vhat")
        nc.vector.memset(vhat[:, :, :, DH], 1.0)
        yt = ytp.tile([P, DT, S], BF16, tag="yt")

        # weight loads: WQ/WK on the ACT hwdge queue; x/WV/WO on SP
        wq_sb = wp.tile([P, DT, D], F32R, tag="w", name="wq_sb")
        nc.scalar.dma_start(wq_sb[:], wq_d.rearrange("(t p) n -> p t n", p=P))
        wk_sb = wp.tile([P, DT, D], F32R, tag="w", name="wk_sb")
        nc.scalar.dma_start(wk_sb[:], wk_d.rearrange("(t p) n -> p t n", p=P))

        # ---- phases A-D: x^T, Q^T, K^T, V ----
        with ExitStack() as es1:
            xtp = es1.enter_context(tc.tile_pool(name="xt", bufs=1))
            xin = es1.enter_context(tc.tile_pool(name="xin", bufs=2))
            pst = es1.enter_context(
                tc.tile_pool(name="pst", bufs=3, space="PSUM"))
            trp = es1.enter_context(
                tc.tile_pool(name="trp", bufs=2, space="PSUM"))

            xT = xtp.tile([P, DT, S], F32R, tag="xT")
            for i in range(ST):
                x_t = xin.tile([P, D], F32, tag="x")
                nc.sync.dma_start(x_t[:], x_d[i * P:(i + 1) * P, :])
                for j in range(DT):
                    tp = trp.tile([P, P], F32, tag="tr")
                    nc.tensor.transpose(
                        tp[:], x_t[:, j * P:(j + 1) * P], ident[:])
                    nc.vector.tensor_copy(
                        xT[:, j, i * P:(i + 1) * P], tp[:])

            wv_sb = wp.tile([P, DT, D], F32R, tag="w", name="wv_sb")
            nc.sync.dma_start(
                wv_sb[:], wv_d.rearrange("(t p) n -> p t n", p=P))
            wo_sb = wp.tile([P, DT, D], F32R, tag="w", name="wo_sb")
            nc.sync.dma_start(
                wo_sb[:], wo_d.rearrange("(t p) n -> p t n", p=P))

            # B/C: Q^T and K^T (ACT applies the per-partition bias, bf16 out)
            for (w_sb, b_sb, dst) in ((wq_sb, bq_sb, qt), (wk_sb, bk_sb, kt)):
                for c in range(DT):
                    ps = pst.tile([P, S], F32, tag="proj")
                    for t in range(DT):
                        for h2 in range(2):
                            nc.tensor.matmul(
                                ps[:, h2 * 512:(h2 + 1) * 512],
                                w_sb[:, t, c * P:(c + 1) * P],
                                xT[:, t, h2 * 512:(h2 + 1) * 512],
                                start=(t == 0), stop=(t == DT - 1))
                    nc.vector.tensor_scalar_add(
                        dst[:, c, :], ps[:], b_sb[:, c:c + 1])

            # D: V with bias folded in as K=1 matmul; bf16 head-strided store
            for c in range(ST):
                ps = pst.tile([P, S], F32, tag="proj")
                for t in range(DT):
                    for h2 in range(2):
                        nc.tensor.matmul(
                            ps[:, h2 * 512:(h2 + 1) * 512],
                            xT[:, t, c * P:(c + 1) * P],
                            wv_sb[:, t, h2 * 512:(h2 + 1) * 512],
                            start=(t == 0), stop=False)
                for h2 in range(2):
                    nc.tensor.matmul(
                        ps[:, h2 * 512:(h2 + 1) * 512],
                        onesr[0:1, 0:P],
                        bv_row[0:1, h2 * 512:(h2 + 1) * 512],
                        start=False, stop=True)
                nc.vector.tensor_copy(
                    vhat[:, c, :, 0:DH],
                    ps[:].rearrange("p (h e) -> p h e", h=H))

        # u = mean_k V, w0 = u @ WO (for the masked-query fixup)
        with tc.tile_pool(name="psu", bufs=2, space="PSUM") as psu, \
             tc.tile_pool(name="psw", bufs=1, space="PSUM") as psw:
            for t in range(DT):
                up = psu.tile([P, 1], F32, tag="u")
                for kk in range(ST):
                    nc.tensor.matmul(
                        up[0:DH, :], vhat[:, kk, 2 * t, 0:DH],
                        onek[:], start=(kk == 0), stop=(kk == ST - 1))
                for kk in range(ST):
                    nc.tensor.matmul(
                        up[DH:P, :], vhat[:, kk, 2 * t + 1, 0:DH],
                        onek[:], start=(kk == 0), stop=(kk == ST - 1),
                        tile_position=(0, DH))
                nc.vector.tensor_copy(ucol[:, t:t + 1], up[:])
            wps = psw.tile([1, D], F32, tag="w0ps")
            for t in range(DT):
                for h2 in range(2):
                    nc.tensor.matmul(
                        wps[0:1, h2 * 512:(h2 + 1) * 512],
                        ucol[:, t:t + 1],
                        wo_sb[:, t, h2 * 512:(h2 + 1) * 512],
                        start=(t == 0), stop=(t == DT - 1))
            nc.vector.tensor_copy(w0_row[:], wps[:])
            nc.vector.tensor_copy(w0_b[:], wps[:])

        # WO -> bf16 for the output GEMM (FWL-eligible weight loads)
        wobp = tc.tile_pool(name="wob", bufs=1)
        es0.callback(lambda: wobp.__exit__(None, None, None))
        wob_pool = wobp.__enter__()
        wob = wob_pool.tile([P, DT, D], BF16, tag="wob")
        for t in range(DT):
            nc.vector.tensor_copy(wob[:, t, :], wo_sb[:, t, :])

        # ---- phase E: attention + phase F: output GEMM, interleaved ----
        with ExitStack() as es2:
            stp = es2.enter_context(
                tc.tile_pool(name="stp", bufs=2, space="PSUM"))
            avp = es2.enter_context(
                tc.tile_pool(name="avp", bufs=2, space="PSUM"))
            fpo = es2.enter_context(
                tc.tile_pool(name="fpo", bufs=2, space="PSUM"))
            epool = es2.enter_context(tc.tile_pool(name="ep", bufs=3))
            avsb = es2.enter_context(tc.tile_pool(name="avsb", bufs=2))
            rcp = es2.enter_context(tc.tile_pool(name="rcp", bufs=1))
            scrp = es2.enter_context(tc.tile_pool(name="scr", bufs=1))
            outp = es2.enter_context(tc.tile_pool(name="outp", bufs=1))

            ob_cur = {}

            def emit_f_unit(c, h2):
                po = fpo.tile([P, 512], F32, tag="o")
                for t in range(DT):
                    nc.tensor.matmul(
                        po[:], yt[:, t, c * P:(c + 1) * P],
                        wob[:, t, h2 * 512:(h2 + 1) * 512],
                        start=(t == 0), stop=False)
                nc.tensor.matmul(
                    po[:], onesb[0:1, 0:P],
                    bo_b[0:1, h2 * 512:(h2 + 1) * 512],
                    start=False, stop=False)
                nc.tensor.matmul(
                    po[:], mf_b[0:1, c * P:(c + 1) * P],
                    w0_b[0:1, h2 * 512:(h2 + 1) * 512],
                    start=False, stop=True)
                if h2 == 0:
                    ob_cur[c] = outp.tile([P, S], F32, tag="osb",
                                          name=f"ob_{c}")
                ob = ob_cur[c]
                nc.vector.tensor_copy(
                    ob[:, h2 * 512:(h2 + 1) * 512], po[:])
                if h2 == 1:
                    nc.sync.dma_start(
                        out_d[c * P:(c + 1) * P, :], ob[:])

            # deferred normalize tail (v1-style ops): emitted mid-way through
            # the NEXT pair's kk loop so PE never head-of-line-blocks
            pending = [None]

            def emit_norm_tail(avs, pr, qs, qc):
                rc64 = rcp.tile([DH + 1, 2, 512], F32R, tag="rc64",
                                name=f"rc64_{qc}_{pr}")
                with nc.allow_low_precision(reason="softmax recip"):
                    nc.vector.reciprocal(
                        rc64[DH:DH + 1, :, :], avs[DH:DH + 1, :, :])
                nc.vector.tensor_tensor(
                    rc64[DH:DH + 1, :, :], rc64[DH:DH + 1, :, :],
                    omq64[DH:DH + 1, :, qs],
                    mybir.AluOpType.mult)
                for sub in range(2):
                    bc = fpo.tile([DH, 512], F32, tag="o",
                                  name=f"bc_{qc}_{pr}_{sub}")
                    nc.tensor.matmul(
                        bc[:], ones64[DH:DH + 1, 0:DH],
                        rc64[DH:DH + 1, sub, :],
                        start=True, stop=True)
                    if sub == 0:
                        nc.vector.tensor_tensor(
                            yt[0:DH, pr, qs], avs[0:DH, 0, :], bc[:],
                            mybir.AluOpType.mult)
                    else:
                        scr = scrp.tile([DH, 512], BF16, tag="scr",
                                        name=f"scr_{qc}_{pr}")
                        nc.vector.tensor_tensor(
                            scr[:], avs[0:DH, 1, :], bc[:],
                            mybir.AluOpType.mult)
                        nc.gpsimd.dma_start(yt[DH:P, pr, qs], scr[:])

            f_queue = []
            for qc in range(2):
                qs = slice(qc * 512, (qc + 1) * 512)
                for pr in range(H // 2):
                    av0 = avp.tile([DH + 1, 512], F32, tag="av",
                                   name=f"av0_{qc}_{pr}")
                    av1 = avp.tile([DH + 1, 512], F32, tag="av",
                                   name=f"av1_{qc}_{pr}")
                    for kk in range(ST):
                        st = stp.tile([P, S], F32, tag="st")
                        nc.tensor.matmul(
                            st[:, 0:512],
                            kt[0:DH, pr, kk * P:(kk + 1) * P],
                            qt[0:DH, pr, qs],
                            start=True, stop=True)
                        nc.tensor.matmul(
                            st[:, 512:1024],
                            kt[DH:P, pr, kk * P:(kk + 1) * P],
                            qt[DH:P, pr, qs],
                            start=True, stop=True)
                        e_t = epool.tile([P, S], BF16, tag="E",
                                         name=f"e_{qc}_{pr}_{kk}")
                        nc.scalar.activation(
                            e_t[:], st[:], AF.Exp,
                            bias=mvec[:, kk:kk + 1], scale=0.125)
                        nc.tensor.matmul(
                            av0[:], vhat[:, kk, 2 * pr, :],
                            e_t[:, 0:512],
                            start=(kk == 0), stop=(kk == ST - 1))
                        nc.tensor.matmul(
                            av1[:], vhat[:, kk, 2 * pr + 1, :],
                            e_t[:, 512:1024],
                            start=(kk == 0), stop=(kk == ST - 1))
                        if kk == 2:
                            if pending[0] is not None:
                                emit_norm_tail(*pending[0])
                                pending[0] = None
                            if f_queue:
                                emit_f_unit(*f_queue.pop(0))
                    # stage AV to SBUF, free PSUM
                    avs = avsb.tile([DH + 1, 2, 512], F32, tag="avs",
                                    name=f"avs_{qc}_{pr}")
                    nc.vector.tensor_copy(avs[:, 0, :], av0[:])
                    nc.vector.tensor_copy(avs[:, 1, :], av1[:])
                    pending[0] = (avs, pr, qs, qc)
                if qc == 0:
                    f_queue = [(c, h2) for c in range(4) for h2 in range(2)]
            if pending[0] is not None:
                emit_norm_tail(*pending[0])
                pending[0] = None
            for cu in f_queue:
                emit_f_unit(*cu)
            for c in range(4, 8):
                for h2 in range(2):
                    emit_f_unit(c, h2)


_cached = {}


def kernel(**inputs):
    ins = {k: np.asarray(v) for k, v in inputs.items()}
    x = ins["x"].astype(np.float32)            # [B, S, D]
    mask = ins["mask"].astype(bool)            # [B, S]
    if "nc" not in _cached:
        _cached["nc"] = _build()
    nc = _cached["nc"]

    mask_f = mask.astype(np.float32)
    weights = {k: np.ascontiguousarray(ins[k].astype(np.float32))
               for k in ("WQ", "WK", "WV", "WO", "bQ", "bK", "bV", "bO")}
    in_maps = []
    for b in range(N_CORES):
        m = dict(weights)
        m["x"] = np.ascontiguousarray(x[b])
        m["mvec"] = np.ascontiguousarray(NEG * mask_f[b])
        m["onem"] = np.ascontiguousarray(1.0 - mask_f[b])
        m["maskf"] = np.ascontiguousarray(mask_f[b])
        in_maps.append(m)

    res = run_bass_kernel_spmd(nc, in_maps, core_ids=list(range(N_CORES)))
    return np.stack([r["out"] for r in res.results], axis=0)
